# revision 13
# baseline (speedup 1.0000x reference)
"""Pointer-net additive-attention scorer on 8 Trainium2 NeuronCores.

score[b,d,m] = sum_h w[h] * tanh(pm[b,m,h] + pd[b,d,h]);  out = masked log_softmax.

Sharding: data-parallel over batch (B=8 -> 8 cores). Per core:
  - pm_T [h,m] = W_mem @ mem_full^T   (PE, bf16, h on partitions in 8 chunks of 128)
  - pd_T [h,d] = W_dec @ dec^T (+ b_mem + b_dec folded in), stored as duplicated pairs
  - one DVE tensor_tensor per d-group broadcasts pm over d and pd over m
    (pair-packed pd keeps the innermost step=1 so bf16 2x mode engages)
  - ACT computes tanh on [128, G*514] tiles (the throughput floor: 1 elem/cycle/lane)
  - PE reduces over h: sliding one-hot window of w -> row d of a [32,512] PSUM tile
    for m<512, plus one strided-rhs gather matmul per group for the last column
  - masked log-softmax along m with exact -inf at masked positions
Host side only reshapes/transposes/casts inputs and merges the boolean masks.
"""

import os
import sys

sys.path.insert(0, "/opt/trn_rl_repo")

import numpy as np
import ml_dtypes
from contextlib import ExitStack

from concourse import bass, bacc, tile, mybir
from concourse.bass_utils import run_bass_kernel_spmd

BF16 = ml_dtypes.bfloat16

B = 8
MEM_LEN = 512
D = 32            # decoder steps
K = 512           # input hidden (MEM_H == DEC_H)
H = 1024          # projection hidden
M1 = MEM_LEN + 1  # 513 = terminate slot + mem positions
MPAD = M1 + 1     # 514, even for DVE packing
NCH = H // 128    # 8 h-chunks
NKB = K // 128    # 4 contraction chunks
G = 16            # d's per tanh instruction / per DVE group-add
NEG_BIG = -3.0e38

AF = mybir.ActivationFunctionType
dt = mybir.dt
ALU = mybir.AluOpType

LAST_RESULT = None
_CACHED_NC = None


def _build_bass():
    nc = bacc.Bacc("TRN2", target_bir_lowering=False, debug=False)

    # Per-core inputs (host pre-transposed / pre-cast).
    memT = nc.declare_dram_parameter("memT", [128, NKB * M1], dt.bfloat16, isOutput=False)
    decT = nc.declare_dram_parameter("decT", [128, NKB * D], dt.bfloat16, isOutput=False)
    WmT = nc.declare_dram_parameter("WmT", [NCH, 128, NKB * 128], dt.bfloat16, isOutput=False)
    WdT = nc.declare_dram_parameter("WdT", [NCH, 128, NKB * 128], dt.bfloat16, isOutput=False)
    bsum = nc.declare_dram_parameter("bsum", [128, NCH], dt.float32, isOutput=False)
    ZW = nc.declare_dram_parameter("ZW", [128, NCH * 63], dt.bfloat16, isOutput=False)
    maskB = nc.declare_dram_parameter("maskB", [D, M1], dt.uint8, isOutput=False)
    eye = nc.declare_dram_parameter("eye", [D, D], dt.bfloat16, isOutput=False)
    out = nc.declare_dram_parameter("out", [D, M1], dt.float32, isOutput=True)

    with tile.TileContext(nc) as tc, ExitStack() as ctx:
        consts = ctx.enter_context(tc.tile_pool(name="consts", bufs=1))
        targ_pool = ctx.enter_context(tc.tile_pool(name="targ", bufs=2))
        tout_pool = ctx.enter_context(tc.tile_pool(name="tout", bufs=3))
        ep_pool = ctx.enter_context(tc.tile_pool(name="ep", bufs=1))
        pm_psum = ctx.enter_context(tc.tile_pool(name="pmps", bufs=2, space="PSUM"))
        pd_psum = ctx.enter_context(tc.tile_pool(name="pdps", bufs=1, space="PSUM"))
        sc_psum = ctx.enter_context(tc.tile_pool(name="scps", bufs=1, space="PSUM"))
        s5_psum = ctx.enter_context(tc.tile_pool(name="s5ps", bufs=1, space="PSUM"))

        WmT_sb = [consts.tile([128, NKB * 128], dt.bfloat16, tag=f"WmT{c}", name=f"WmT{c}") for c in range(NCH)]
        WdT_sb = [consts.tile([128, NKB * 128], dt.bfloat16, tag=f"WdT{c}", name=f"WdT{c}") for c in range(NCH)]

        def load_w(c):
            nc.sync.dma_start(WdT_sb[c][:], WdT[c])
            nc.sync.dma_start(WmT_sb[c][:], WmT[c])

        # ---- early loads (needed by chunk 0) ----
        decT_sb = consts.tile([128, NKB * D], dt.bfloat16, tag="decT")
        nc.sync.dma_start(decT_sb[:], decT[:, :])
        bsum_sb = consts.tile([128, NCH], dt.float32, tag="bsum")
        nc.sync.dma_start(bsum_sb[:], bsum[:, :])
        load_w(0)
        memT_sb = consts.tile([128, NKB * M1], dt.bfloat16, tag="memT")
        for kb in range(NKB):
            nc.sync.dma_start(memT_sb[:, bass.ts(kb, M1)], memT[:, bass.ts(kb, M1)])
        load_w(1)
        ZW_sb = consts.tile([128, NCH * 63], dt.bfloat16, tag="ZW")
        nc.gpsimd.dma_start(ZW_sb[:], ZW[:, :])
        maskB_sb = consts.tile([D, M1], dt.uint8, tag="maskB")
        nc.gpsimd.dma_start(maskB_sb[:], maskB[:, :])
        eye_sb = consts.tile([D, D], dt.bfloat16, tag="eye")
        nc.gpsimd.dma_start(eye_sb[:], eye[:, :])

        # ---- mask prep (float additive masks) ----
        maskF = ep_pool.tile([D, M1], dt.bfloat16, tag="maskF")  # 0 or -3e38
        nc.vector.tensor_scalar_mul(maskF[:], maskB_sb[:], NEG_BIG)
        mask2 = ep_pool.tile([D, M1], dt.float32, tag="mask2")  # 0 or -inf
        nc.vector.tensor_scalar_mul(mask2[:], maskF[:], 2.0)

        # pd values stored as duplicated pairs (pd[d] at cols 2d and 2d+1)
        # so the group-add's in1 AP keeps an innermost step-1 pair dim.
        pdb = consts.tile([128, NCH * D * 2], dt.bfloat16, tag="pdb")
        pm_sb = consts.tile([128, NCH * MPAD], dt.bfloat16, tag="pm")

        def compute_chunk(c):
            """pd_T and pm_T for h-chunk c."""
            ps = pd_psum.tile([128, D], dt.float32, tag="pdps")
            for kb in range(NKB):
                nc.tensor.matmul(
                    ps[:],
                    WdT_sb[c][:, bass.ts(kb, 128)],
                    decT_sb[:, bass.ts(kb, D)],
                    start=(kb == 0),
                    stop=(kb == NKB - 1),
                )
            # psum -> sbuf pairs with per-partition (+b_mem+b_dec) fold
            dup_out = pdb[:, bass.ts(c, 2 * D)].rearrange("p (d two) -> p d two", two=2)
            dup_in = ps[:].unsqueeze(2).broadcast_to((128, D, 2))
            nc.vector.tensor_scalar_add(dup_out, dup_in, bsum_sb[:, c : c + 1])

            pp = pm_psum.tile([128, M1], dt.float32, tag="pmps")
            for kb in range(NKB):
                lhs = WmT_sb[c][:, bass.ts(kb, 128)]
                nc.tensor.matmul(
                    pp[:, 0:512],
                    lhs,
                    memT_sb[:, kb * M1 : kb * M1 + 512],
                    start=(kb == 0),
                    stop=(kb == NKB - 1),
                    skip_group_check=True,
                )
                nc.tensor.matmul(
                    pp[:, 512:513],
                    lhs,
                    memT_sb[:, kb * M1 + 512 : kb * M1 + 513],
                    start=(kb == 0),
                    stop=(kb == NKB - 1),
                    skip_group_check=True,
                )
            nc.vector.tensor_copy(pm_sb[:, c * MPAD : c * MPAD + M1], pp[:])
            nc.vector.memset(pm_sb[:, c * MPAD + M1 : (c + 1) * MPAD], 0.0)

        compute_chunk(0)

        # ---- main loop: tanh(pm + pd) and weighted reduce over h ----
        # m < 512 goes through sliding one-hot matmuls into score_ps[d, m];
        # the last column (m=512) is gathered per group into s512_ps[0, d].
        score_ps = sc_psum.tile([D, 512], dt.float32, tag="score")
        s512_ps = s5_psum.tile([1, D], dt.float32, tag="s512")
        first = True
        # taper the final chunk so the post-tanh PE tail is short
        groups = [(0, 16), (16, 16)]
        first_groups = [(0, 4), (4, 4), (8, 8), (16, 8), (24, 8)]
        last_groups = [(0, 16), (16, 8), (24, 4), (28, 4)]
        for c in range(NCH):
            pm_c = pm_sb[:, c * MPAD : (c + 1) * MPAD]
            glist = (
                first_groups if c == 0
                else last_groups if c == NCH - 1
                else groups
            )
            for gi, (d0, gsz) in enumerate(glist):
                targ = targ_pool.tile([128, G * MPAD], dt.bfloat16, tag="targ")
                in0 = (
                    pm_c.unsqueeze(1)
                    .broadcast_to((128, gsz, MPAD))
                    .rearrange("p d (a two) -> p d a two", two=2)
                )
                in1 = (
                    pdb[:, c * 2 * D + 2 * d0 : c * 2 * D + 2 * (d0 + gsz)]
                    .rearrange("p (d two) -> p d two", two=2)
                    .unsqueeze(2)
                    .broadcast_to((128, gsz, MPAD // 2, 2))
                )
                tadd = targ[:, 0 : gsz * MPAD].rearrange(
                    "p (d a two) -> p d a two", d=gsz, two=2
                )
                nc.vector.tensor_add(tadd, in0, in1)

                tout = tout_pool.tile([128, G * MPAD], dt.bfloat16, tag="tout")
                nc.scalar.activation(tout[:, 0 : gsz * MPAD], targ[:, 0 : gsz * MPAD], AF.Tanh)
                last = c == NCH - 1 and gi == len(glist) - 1
                if last:
                    g_rhs0 = (
                        tout[:, 0 : gsz * MPAD]
                        .rearrange("p (d m) -> p d m", d=gsz)[:, :, 512:513]
                        .squeeze(2)
                    )
                    nc.tensor.matmul(
                        s512_ps[0:1, d0 : d0 + gsz],
                        ZW_sb[:, c * 63 + 31 : c * 63 + 32],
                        g_rhs0,
                        start=False,
                        stop=True,
                        skip_group_check=True,
                    )
                for j in range(gsz):
                    d = d0 + j
                    lhs = ZW_sb[:, c * 63 + 31 - d : c * 63 + 63 - d]
                    nc.tensor.matmul(
                        score_ps[:, 0:512],
                        lhs,
                        tout[:, j * MPAD : j * MPAD + 512],
                        start=first,
                        stop=(last and j == gsz - 1),
                        skip_group_check=True,
                    )
                    first = False
                # gather the m=512 column of every d in this group at once
                if not last:
                    g_rhs = (
                        tout[:, 0 : gsz * MPAD]
                        .rearrange("p (d m) -> p d m", d=gsz)[:, :, 512:513]
                        .squeeze(2)
                    )
                    nc.tensor.matmul(
                        s512_ps[0:1, d0 : d0 + gsz],
                        ZW_sb[:, c * 63 + 31 : c * 63 + 32],
                        g_rhs,
                        start=(c == 0 and gi == 0),
                        stop=False,
                        skip_group_check=True,
                    )
                if c == 0 and gi == 0:
                    # fold the additive -3e38 mask into the PSUM accumulators
                    nc.tensor.matmul(
                        score_ps[:, 0:512], eye_sb[:], maskF[:, 0:512],
                        start=False, stop=False, skip_group_check=True,
                    )
                    nc.tensor.matmul(
                        s512_ps[0:1, :], maskF[:, 512:513], eye_sb[:],
                        start=False, stop=False, skip_group_check=True,
                    )
                # pipeline the next chunk's projections behind this chunk's tanh
                if gi == (4 if c == 0 else 0):
                    if c + 2 < NCH:
                        load_w(c + 2)
                    if c + 1 < NCH:
                        compute_chunk(c + 1)

        # ---- masked log-softmax along m (mask already folded into PSUM) ----
        # transpose the gathered last column [1,32] -> [32,1]
        tr = ep_pool.tile([32, 32], dt.float32, tag="tr")
        nc.vector.tensor_copy(tr[0:1, :], s512_ps[:])
        tr2 = ep_pool.tile([32, 32], dt.float32, tag="tr2")
        nc.vector.transpose(tr2[:], tr[:])

        nmx = ep_pool.tile([D, 1], dt.float32, tag="nmx")
        nc.vector.tensor_reduce(
            nmx[:], score_ps[:], axis=mybir.AxisListType.X, op=ALU.max, negate=True
        )
        E = ep_pool.tile([D, 512], dt.float32, tag="E")
        Z1 = ep_pool.tile([D, 1], dt.float32, tag="Z1")
        nc.scalar.activation(E[:], score_ps[:], AF.Exp, bias=nmx[:, 0:1], accum_out=Z1[:])
        E2 = ep_pool.tile([D, 1], dt.float32, tag="E2")
        nc.scalar.activation(E2[:], tr2[:, 0:1], AF.Exp, bias=nmx[:, 0:1])
        Z = ep_pool.tile([D, 1], dt.float32, tag="Z")
        nc.vector.tensor_add(Z[:], Z1[:], E2[:])
        # ln(Z) via exponent/mantissa split + degree-5 polynomial on [1,2).
        # Z = sum(exp(S-max)) is always in [1, 514*e^5], so e in [0,17], m in [1,2).
        LN_C = [-1.9367597429421655, 3.514087297000313, -2.440029762614545,
                1.1160900268324192, -0.2838268477821446, 0.030449004538674287]
        zb = Z[:].bitcast(dt.uint32)
        eb = ep_pool.tile([D, 1], dt.uint32, tag="eb")
        nc.vector.tensor_scalar(
            eb[:], zb, 23, None, op0=ALU.logical_shift_right
        )
        ef = ep_pool.tile([D, 1], dt.float32, tag="ef")
        nc.vector.tensor_copy(ef[:], eb[:])  # biased exponent as f32
        mb = ep_pool.tile([D, 1], dt.uint32, tag="mb")
        nc.vector.tensor_scalar(
            mb[:], zb, 0x007FFFFF, 0x3F800000,
            op0=ALU.bitwise_and, op1=ALU.bitwise_or,
        )
        mant = mb[:].bitcast(dt.float32)
        acc = ep_pool.tile([D, 1], dt.float32, tag="acc")
        nc.vector.tensor_scalar(acc[:], mant, LN_C[5], None, op0=ALU.mult)
        for k in (4, 3, 2, 1):
            nc.vector.scalar_tensor_tensor(
                acc[:], acc[:], LN_C[k], mant, ALU.add, ALU.mult
            )
        off = ep_pool.tile([D, 1], dt.float32, tag="off")
        # off = e*ln2 + acc
        nc.vector.scalar_tensor_tensor(
            off[:], ef[:], 0.6931471805599453, acc[:], ALU.mult, ALU.add
        )
        # off2 = (off + c0 - 127*ln2) - nmx = lnZ + mx
        off2 = ep_pool.tile([D, 1], dt.float32, tag="off2")
        nc.vector.scalar_tensor_tensor(
            off2[:], off[:], LN_C[0] - 127 * 0.6931471805599453, nmx[:],
            ALU.add, ALU.subtract
        )
        res2 = ep_pool.tile([D, M1], dt.float32, tag="res2")
        # res2 = (score - off2) + mask2  (exact -inf at masked positions)
        nc.vector.scalar_tensor_tensor(
            res2[:, 0:512], score_ps[:], off2[:, 0:1], mask2[:, 0:512],
            ALU.subtract, ALU.add
        )
        nc.vector.scalar_tensor_tensor(
            res2[:, 512:513], tr2[:, 0:1], off2[:, 0:1], mask2[:, 512:513],
            ALU.subtract, ALU.add
        )
        nc.sync.dma_start(out[:, 0:256], res2[:, 0:256])
        nc.gpsimd.dma_start(out[:, 256:M1], res2[:, 256:M1])

    nc.compile()
    return nc


def _get_nc():
    global _CACHED_NC
    if _CACHED_NC is None:
        _CACHED_NC = _build_bass()
    return _CACHED_NC


def kernel(
    mem,
    dec_hid,
    mem_mask,
    dec_mask,
    dup_mask,
    terminate_state,
    W_mem,
    b_mem,
    W_dec,
    b_dec,
    w_score,
    b_score,
):
    global LAST_RESULT
    mem = np.asarray(mem, np.float32)
    dec_hid = np.asarray(dec_hid, np.float32)
    mem_mask = np.asarray(mem_mask, bool)
    dec_mask = np.asarray(dec_mask, bool)
    dup_mask = np.asarray(dup_mask, bool)
    terminate_state = np.asarray(terminate_state, np.float32)
    W_mem = np.asarray(W_mem, np.float32)
    b_mem = np.asarray(b_mem, np.float32)
    W_dec = np.asarray(W_dec, np.float32)
    b_dec = np.asarray(b_dec, np.float32)
    w_score = np.asarray(w_score, np.float32)
    # b_score shifts every logit equally -> cancels in log_softmax; ignore it.

    b = mem.shape[0]

    # Shared (per-core identical) tensors, in the exact SBUF tile layouts.
    WmT = np.ascontiguousarray(
        W_mem.T.reshape(NKB, 128, NCH, 128).transpose(2, 1, 0, 3).reshape(NCH, 128, NKB * 128)
    ).astype(BF16)
    WdT = np.ascontiguousarray(
        W_dec.T.reshape(NKB, 128, NCH, 128).transpose(2, 1, 0, 3).reshape(NCH, 128, NKB * 128)
    ).astype(BF16)
    bsum = np.ascontiguousarray((b_mem + b_dec).reshape(NCH, 128).T).astype(np.float32)
    EYE = np.eye(D, dtype=np.float32).astype(BF16)
    ZW = np.zeros((128, NCH * 63), BF16)
    for c in range(NCH):
        ZW[:, c * 63 + 31] = w_score[c * 128 : (c + 1) * 128].astype(BF16)

    # Merged boolean mask: True -> -inf.
    mem_mask_full = np.concatenate(
        [np.zeros((b, 1), bool), mem_mask], axis=1
    )  # [B, 513]
    full_mask = (dup_mask & ~dec_mask[:, :, None]) | mem_mask_full[:, None, :]

    in_maps = []
    for i in range(b):
        memT_full = np.empty((K, M1), np.float32)
        memT_full[:, 0] = terminate_state[0]
        memT_full[:, 1:] = mem[i].T
        memTr = memT_full.reshape(NKB, 128, M1).transpose(1, 0, 2).reshape(128, NKB * M1)
        decTr = dec_hid[i].T.reshape(NKB, 128, D).transpose(1, 0, 2).reshape(128, NKB * D)
        in_maps.append(
            {
                "memT": np.ascontiguousarray(memTr).astype(BF16),
                "decT": np.ascontiguousarray(decTr).astype(BF16),
                "WmT": WmT,
                "WdT": WdT,
                "bsum": bsum,
                "ZW": ZW,
                "maskB": full_mask[i].astype(np.uint8),
                "eye": EYE,
            }
        )

    nc = _get_nc()
    trace = bool(os.environ.get("PTR_TRACE"))
    LAST_RESULT = run_bass_kernel_spmd(
        nc, in_maps, list(range(b)), trace=trace
    )
    outs = np.stack([LAST_RESULT.results[i]["out"] for i in range(b)])
    return outs.astype(np.float32)


# revision 14
# speedup vs baseline: 1.0074x; 1.0074x over previous
"""Pointer-net additive-attention scorer on 8 Trainium2 NeuronCores.

score[b,d,m] = sum_h w[h] * tanh(pm[b,m,h] + pd[b,d,h]);  out = masked log_softmax.

Sharding: data-parallel over batch (B=8 -> 8 cores). Per core:
  - pm_T [h,m] = W_mem @ mem_full^T   (PE, bf16, h on partitions in 8 chunks of 128)
  - pd_T [h,d] = W_dec @ dec^T (+ b_mem + b_dec folded in), stored as duplicated pairs
  - one DVE tensor_tensor per d-group broadcasts pm over d and pd over m
    (pair-packed pd keeps the innermost step=1 so bf16 2x mode engages)
  - ACT computes tanh on [128, G*514] tiles (the throughput floor: 1 elem/cycle/lane)
  - PE reduces over h: sliding one-hot window of w -> row d of a [32,512] PSUM tile
    for m<512, plus one strided-rhs gather matmul per group for the last column
  - masked log-softmax along m with exact -inf at masked positions
Host side only reshapes/transposes/casts inputs and merges the boolean masks.
"""

import os
import sys

sys.path.insert(0, "/opt/trn_rl_repo")

import numpy as np
import ml_dtypes
from contextlib import ExitStack

from concourse import bass, bacc, tile, mybir
from concourse.bass_utils import run_bass_kernel_spmd

BF16 = ml_dtypes.bfloat16

B = 8
MEM_LEN = 512
D = 32            # decoder steps
K = 512           # input hidden (MEM_H == DEC_H)
H = 1024          # projection hidden
M1 = MEM_LEN + 1  # 513 = terminate slot + mem positions
MPAD = M1 + 1     # 514, even for DVE packing
NCH = H // 128    # 8 h-chunks
NKB = K // 128    # 4 contraction chunks
G = 16            # d's per tanh instruction / per DVE group-add
NEG_BIG = -3.0e38

AF = mybir.ActivationFunctionType
dt = mybir.dt
ALU = mybir.AluOpType

LAST_RESULT = None
_CACHED_NC = None


def _build_bass():
    nc = bacc.Bacc("TRN2", target_bir_lowering=False, debug=False)

    # Per-core inputs (host pre-transposed / pre-cast).
    memT = nc.declare_dram_parameter("memT", [128, NKB * M1], dt.bfloat16, isOutput=False)
    decT = nc.declare_dram_parameter("decT", [128, NKB * D], dt.bfloat16, isOutput=False)
    WmT = nc.declare_dram_parameter("WmT", [NCH, 128, NKB * 128], dt.bfloat16, isOutput=False)
    WdT = nc.declare_dram_parameter("WdT", [NCH, 128, NKB * 128], dt.bfloat16, isOutput=False)
    bsum = nc.declare_dram_parameter("bsum", [128, NCH], dt.float32, isOutput=False)
    ZW = nc.declare_dram_parameter("ZW", [128, NCH * 63], dt.bfloat16, isOutput=False)
    maskB = nc.declare_dram_parameter("maskB", [D, M1], dt.uint8, isOutput=False)
    eye = nc.declare_dram_parameter("eye", [D, D], dt.bfloat16, isOutput=False)
    out = nc.declare_dram_parameter("out", [D, M1], dt.float32, isOutput=True)

    with tile.TileContext(nc) as tc, ExitStack() as ctx:
        consts = ctx.enter_context(tc.tile_pool(name="consts", bufs=1))
        targ_pool = ctx.enter_context(tc.tile_pool(name="targ", bufs=2))
        tout_pool = ctx.enter_context(tc.tile_pool(name="tout", bufs=3))
        ep_pool = ctx.enter_context(tc.tile_pool(name="ep", bufs=1))
        pm_psum = ctx.enter_context(tc.tile_pool(name="pmps", bufs=2, space="PSUM"))
        pd_psum = ctx.enter_context(tc.tile_pool(name="pdps", bufs=1, space="PSUM"))
        sc_psum = ctx.enter_context(tc.tile_pool(name="scps", bufs=1, space="PSUM"))
        s5_psum = ctx.enter_context(tc.tile_pool(name="s5ps", bufs=1, space="PSUM"))

        WmT_sb = [consts.tile([128, NKB * 128], dt.bfloat16, tag=f"WmT{c}", name=f"WmT{c}") for c in range(NCH)]
        WdT_sb = [consts.tile([128, NKB * 128], dt.bfloat16, tag=f"WdT{c}", name=f"WdT{c}") for c in range(NCH)]

        def load_w(c):
            nc.sync.dma_start(WdT_sb[c][:], WdT[c])
            nc.sync.dma_start(WmT_sb[c][:], WmT[c])

        # ---- early loads (needed by chunk 0) ----
        decT_sb = consts.tile([128, NKB * D], dt.bfloat16, tag="decT")
        nc.sync.dma_start(decT_sb[:], decT[:, :])
        bsum_sb = consts.tile([128, NCH], dt.float32, tag="bsum")
        nc.sync.dma_start(bsum_sb[:], bsum[:, :])
        load_w(0)
        memT_sb = consts.tile([128, NKB * M1], dt.bfloat16, tag="memT")
        for kb in range(NKB):
            nc.sync.dma_start(memT_sb[:, bass.ts(kb, M1)], memT[:, bass.ts(kb, M1)])
        load_w(1)
        ZW_sb = consts.tile([128, NCH * 63], dt.bfloat16, tag="ZW")
        nc.gpsimd.dma_start(ZW_sb[:], ZW[:, :])
        maskB_sb = consts.tile([D, M1], dt.uint8, tag="maskB")
        nc.gpsimd.dma_start(maskB_sb[:], maskB[:, :])
        eye_sb = consts.tile([D, D], dt.bfloat16, tag="eye")
        nc.gpsimd.dma_start(eye_sb[:], eye[:, :])

        # pd values stored as duplicated pairs (pd[d] at cols 2d and 2d+1)
        # so the group-add's in1 AP keeps an innermost step-1 pair dim.
        pdb = consts.tile([128, NCH * D * 2], dt.bfloat16, tag="pdb")
        pm_sb = consts.tile([128, NCH * MPAD], dt.bfloat16, tag="pm")

        def compute_chunk(c):
            """pd_T and pm_T for h-chunk c."""
            ps = pd_psum.tile([128, D], dt.float32, tag="pdps")
            for kb in range(NKB):
                nc.tensor.matmul(
                    ps[:],
                    WdT_sb[c][:, bass.ts(kb, 128)],
                    decT_sb[:, bass.ts(kb, D)],
                    start=(kb == 0),
                    stop=(kb == NKB - 1),
                )
            # psum -> sbuf pairs with per-partition (+b_mem+b_dec) fold
            dup_out = pdb[:, bass.ts(c, 2 * D)].rearrange("p (d two) -> p d two", two=2)
            dup_in = ps[:].unsqueeze(2).broadcast_to((128, D, 2))
            nc.vector.tensor_scalar_add(dup_out, dup_in, bsum_sb[:, c : c + 1])

            pp = pm_psum.tile([128, M1], dt.float32, tag="pmps")
            for kb in range(NKB):
                lhs = WmT_sb[c][:, bass.ts(kb, 128)]
                nc.tensor.matmul(
                    pp[:, 0:512],
                    lhs,
                    memT_sb[:, kb * M1 : kb * M1 + 512],
                    start=(kb == 0),
                    stop=(kb == NKB - 1),
                    skip_group_check=True,
                )
                nc.tensor.matmul(
                    pp[:, 512:513],
                    lhs,
                    memT_sb[:, kb * M1 + 512 : kb * M1 + 513],
                    start=(kb == 0),
                    stop=(kb == NKB - 1),
                    skip_group_check=True,
                )
            nc.vector.tensor_copy(pm_sb[:, c * MPAD : c * MPAD + M1], pp[:])
            nc.vector.memset(pm_sb[:, c * MPAD + M1 : (c + 1) * MPAD], 0.0)

        compute_chunk(0)

        # ---- main loop: tanh(pm + pd) and weighted reduce over h ----
        # m < 512 goes through sliding one-hot matmuls into score_ps[d, m];
        # the last column (m=512) is gathered per group into s512_ps[0, d].
        score_ps = sc_psum.tile([D, 512], dt.float32, tag="score")
        s512_ps = s5_psum.tile([1, D], dt.float32, tag="s512")
        first = True
        # taper the final chunk so the post-tanh PE tail is short
        groups = [(0, 16), (16, 16)]
        first_groups = [(0, 4), (4, 4), (8, 8), (16, 8), (24, 8)]
        last_groups = [(0, 16), (16, 8), (24, 4), (28, 4)]
        for c in range(NCH):
            pm_c = pm_sb[:, c * MPAD : (c + 1) * MPAD]
            glist = (
                first_groups if c == 0
                else last_groups if c == NCH - 1
                else groups
            )
            for gi, (d0, gsz) in enumerate(glist):
                targ = targ_pool.tile([128, G * MPAD], dt.bfloat16, tag="targ")
                in0 = (
                    pm_c.unsqueeze(1)
                    .broadcast_to((128, gsz, MPAD))
                    .rearrange("p d (a two) -> p d a two", two=2)
                )
                in1 = (
                    pdb[:, c * 2 * D + 2 * d0 : c * 2 * D + 2 * (d0 + gsz)]
                    .rearrange("p (d two) -> p d two", two=2)
                    .unsqueeze(2)
                    .broadcast_to((128, gsz, MPAD // 2, 2))
                )
                tadd = targ[:, 0 : gsz * MPAD].rearrange(
                    "p (d a two) -> p d a two", d=gsz, two=2
                )
                nc.vector.tensor_add(tadd, in0, in1)

                tout = tout_pool.tile([128, G * MPAD], dt.bfloat16, tag="tout")
                nc.scalar.activation(tout[:, 0 : gsz * MPAD], targ[:, 0 : gsz * MPAD], AF.Tanh)
                last = c == NCH - 1 and gi == len(glist) - 1
                if last:
                    g_rhs0 = (
                        tout[:, 0 : gsz * MPAD]
                        .rearrange("p (d m) -> p d m", d=gsz)[:, :, 512:513]
                        .squeeze(2)
                    )
                    nc.tensor.matmul(
                        s512_ps[0:1, d0 : d0 + gsz],
                        ZW_sb[:, c * 63 + 31 : c * 63 + 32],
                        g_rhs0,
                        start=False,
                        stop=True,
                        skip_group_check=True,
                    )
                for j in range(gsz):
                    d = d0 + j
                    lhs = ZW_sb[:, c * 63 + 31 - d : c * 63 + 63 - d]
                    nc.tensor.matmul(
                        score_ps[:, 0:512],
                        lhs,
                        tout[:, j * MPAD : j * MPAD + 512],
                        start=first,
                        stop=(last and j == gsz - 1),
                        skip_group_check=True,
                    )
                    first = False
                # gather the m=512 column of every d in this group at once
                if not last:
                    g_rhs = (
                        tout[:, 0 : gsz * MPAD]
                        .rearrange("p (d m) -> p d m", d=gsz)[:, :, 512:513]
                        .squeeze(2)
                    )
                    nc.tensor.matmul(
                        s512_ps[0:1, d0 : d0 + gsz],
                        ZW_sb[:, c * 63 + 31 : c * 63 + 32],
                        g_rhs,
                        start=(c == 0 and gi == 0),
                        stop=False,
                        skip_group_check=True,
                    )
                if c == 2 and gi == 0:
                    # fold the additive -3e38 mask into the PSUM accumulators
                    maskF = ep_pool.tile([D, M1], dt.bfloat16, tag="maskF")
                    nc.vector.tensor_scalar_mul(maskF[:], maskB_sb[:], NEG_BIG)
                    mask2 = ep_pool.tile([D, M1], dt.float32, tag="mask2")
                    nc.vector.tensor_scalar_mul(mask2[:], maskF[:], 2.0)
                    nc.tensor.matmul(
                        score_ps[:, 0:512], eye_sb[:], maskF[:, 0:512],
                        start=False, stop=False, skip_group_check=True,
                    )
                    nc.tensor.matmul(
                        s512_ps[0:1, :], maskF[:, 512:513], eye_sb[:],
                        start=False, stop=False, skip_group_check=True,
                    )
                # pipeline the next chunk's projections behind this chunk's tanh
                if gi == (4 if c == 0 else 0):
                    if c + 2 < NCH:
                        load_w(c + 2)
                    if c + 1 < NCH:
                        compute_chunk(c + 1)

        # ---- masked log-softmax along m (mask already folded into PSUM) ----
        # transpose the gathered last column [1,32] -> [32,1]
        tr = ep_pool.tile([32, 32], dt.float32, tag="tr")
        nc.vector.tensor_copy(tr[0:1, :], s512_ps[:])
        tr2 = ep_pool.tile([32, 32], dt.float32, tag="tr2")
        nc.vector.transpose(tr2[:], tr[:])

        nmx = ep_pool.tile([D, 1], dt.float32, tag="nmx")
        nc.vector.tensor_reduce(
            nmx[:], score_ps[:], axis=mybir.AxisListType.X, op=ALU.max, negate=True
        )
        E = ep_pool.tile([D, 512], dt.float32, tag="E")
        Z1 = ep_pool.tile([D, 1], dt.float32, tag="Z1")
        nc.scalar.activation(E[:], score_ps[:], AF.Exp, bias=nmx[:, 0:1], accum_out=Z1[:])
        E2 = ep_pool.tile([D, 1], dt.float32, tag="E2")
        nc.scalar.activation(E2[:], tr2[:, 0:1], AF.Exp, bias=nmx[:, 0:1])
        Z = ep_pool.tile([D, 1], dt.float32, tag="Z")
        nc.vector.tensor_add(Z[:], Z1[:], E2[:])
        # ln(Z) via exponent/mantissa split + degree-5 polynomial on [1,2).
        # Z = sum(exp(S-max)) is always in [1, 514*e^5], so e in [0,17], m in [1,2).
        LN_C = [-1.9367597429421655, 3.514087297000313, -2.440029762614545,
                1.1160900268324192, -0.2838268477821446, 0.030449004538674287]
        zb = Z[:].bitcast(dt.uint32)
        eb = ep_pool.tile([D, 1], dt.uint32, tag="eb")
        nc.vector.tensor_scalar(
            eb[:], zb, 23, None, op0=ALU.logical_shift_right
        )
        ef = ep_pool.tile([D, 1], dt.float32, tag="ef")
        nc.vector.tensor_copy(ef[:], eb[:])  # biased exponent as f32
        mb = ep_pool.tile([D, 1], dt.uint32, tag="mb")
        nc.vector.tensor_scalar(
            mb[:], zb, 0x007FFFFF, 0x3F800000,
            op0=ALU.bitwise_and, op1=ALU.bitwise_or,
        )
        mant = mb[:].bitcast(dt.float32)
        acc = ep_pool.tile([D, 1], dt.float32, tag="acc")
        nc.vector.tensor_scalar(acc[:], mant, LN_C[5], None, op0=ALU.mult)
        for k in (4, 3, 2, 1):
            nc.vector.scalar_tensor_tensor(
                acc[:], acc[:], LN_C[k], mant, ALU.add, ALU.mult
            )
        off = ep_pool.tile([D, 1], dt.float32, tag="off")
        # off = e*ln2 + acc
        nc.vector.scalar_tensor_tensor(
            off[:], ef[:], 0.6931471805599453, acc[:], ALU.mult, ALU.add
        )
        # off2 = (off + c0 - 127*ln2) - nmx = lnZ + mx
        off2 = ep_pool.tile([D, 1], dt.float32, tag="off2")
        nc.vector.scalar_tensor_tensor(
            off2[:], off[:], LN_C[0] - 127 * 0.6931471805599453, nmx[:],
            ALU.add, ALU.subtract
        )
        res2 = ep_pool.tile([D, M1], dt.float32, tag="res2")
        # res2 = (score - off2) + mask2  (exact -inf at masked positions)
        nc.vector.scalar_tensor_tensor(
            res2[:, 0:512], score_ps[:], off2[:, 0:1], mask2[:, 0:512],
            ALU.subtract, ALU.add
        )
        nc.vector.scalar_tensor_tensor(
            res2[:, 512:513], tr2[:, 0:1], off2[:, 0:1], mask2[:, 512:513],
            ALU.subtract, ALU.add
        )
        nc.sync.dma_start(out[:, 0:256], res2[:, 0:256])
        nc.gpsimd.dma_start(out[:, 256:M1], res2[:, 256:M1])

    nc.compile()
    return nc


def _get_nc():
    global _CACHED_NC
    if _CACHED_NC is None:
        _CACHED_NC = _build_bass()
    return _CACHED_NC


def kernel(
    mem,
    dec_hid,
    mem_mask,
    dec_mask,
    dup_mask,
    terminate_state,
    W_mem,
    b_mem,
    W_dec,
    b_dec,
    w_score,
    b_score,
):
    global LAST_RESULT
    mem = np.asarray(mem, np.float32)
    dec_hid = np.asarray(dec_hid, np.float32)
    mem_mask = np.asarray(mem_mask, bool)
    dec_mask = np.asarray(dec_mask, bool)
    dup_mask = np.asarray(dup_mask, bool)
    terminate_state = np.asarray(terminate_state, np.float32)
    W_mem = np.asarray(W_mem, np.float32)
    b_mem = np.asarray(b_mem, np.float32)
    W_dec = np.asarray(W_dec, np.float32)
    b_dec = np.asarray(b_dec, np.float32)
    w_score = np.asarray(w_score, np.float32)
    # b_score shifts every logit equally -> cancels in log_softmax; ignore it.

    b = mem.shape[0]

    # Shared (per-core identical) tensors, in the exact SBUF tile layouts.
    WmT = np.ascontiguousarray(
        W_mem.T.reshape(NKB, 128, NCH, 128).transpose(2, 1, 0, 3).reshape(NCH, 128, NKB * 128)
    ).astype(BF16)
    WdT = np.ascontiguousarray(
        W_dec.T.reshape(NKB, 128, NCH, 128).transpose(2, 1, 0, 3).reshape(NCH, 128, NKB * 128)
    ).astype(BF16)
    bsum = np.ascontiguousarray((b_mem + b_dec).reshape(NCH, 128).T).astype(np.float32)
    EYE = np.eye(D, dtype=np.float32).astype(BF16)
    ZW = np.zeros((128, NCH * 63), BF16)
    for c in range(NCH):
        ZW[:, c * 63 + 31] = w_score[c * 128 : (c + 1) * 128].astype(BF16)

    # Merged boolean mask: True -> -inf.
    mem_mask_full = np.concatenate(
        [np.zeros((b, 1), bool), mem_mask], axis=1
    )  # [B, 513]
    full_mask = (dup_mask & ~dec_mask[:, :, None]) | mem_mask_full[:, None, :]

    in_maps = []
    for i in range(b):
        memT_full = np.empty((K, M1), np.float32)
        memT_full[:, 0] = terminate_state[0]
        memT_full[:, 1:] = mem[i].T
        memTr = memT_full.reshape(NKB, 128, M1).transpose(1, 0, 2).reshape(128, NKB * M1)
        decTr = dec_hid[i].T.reshape(NKB, 128, D).transpose(1, 0, 2).reshape(128, NKB * D)
        in_maps.append(
            {
                "memT": np.ascontiguousarray(memTr).astype(BF16),
                "decT": np.ascontiguousarray(decTr).astype(BF16),
                "WmT": WmT,
                "WdT": WdT,
                "bsum": bsum,
                "ZW": ZW,
                "maskB": full_mask[i].astype(np.uint8),
                "eye": EYE,
            }
        )

    nc = _get_nc()
    trace = bool(os.environ.get("PTR_TRACE"))
    LAST_RESULT = run_bass_kernel_spmd(
        nc, in_maps, list(range(b)), trace=trace
    )
    outs = np.stack([LAST_RESULT.results[i]["out"] for i in range(b)])
    return outs.astype(np.float32)


# revision 15
# speedup vs baseline: 1.0204x; 1.0129x over previous
"""Pointer-net additive-attention scorer on 8 Trainium2 NeuronCores.

score[b,d,m] = sum_h w[h] * tanh(pm[b,m,h] + pd[b,d,h]);  out = masked log_softmax.

Sharding: data-parallel over batch (B=8 -> 8 cores). Per core:
  - pm_T [h,m] = W_mem @ mem_full^T   (PE, bf16, h on partitions in 8 chunks of 128)
  - pd_T [h,d] = W_dec @ dec^T (+ b_mem + b_dec folded in), stored as duplicated pairs
  - one DVE tensor_tensor per d-group broadcasts pm over d and pd over m
    (pair-packed pd keeps the innermost step=1 so bf16 2x mode engages)
  - ACT computes tanh on [128, G*514] tiles (the throughput floor: 1 elem/cycle/lane)
  - PE reduces over h: sliding one-hot window of w -> row d of a [32,512] PSUM tile
    for m<512, plus one strided-rhs gather matmul per group for the last column
  - masked log-softmax along m with exact -inf at masked positions
Host side only reshapes/transposes/casts inputs and merges the boolean masks.
"""

import os
import sys

sys.path.insert(0, "/opt/trn_rl_repo")

import numpy as np
import ml_dtypes
from contextlib import ExitStack

from concourse import bass, bacc, tile, mybir
from concourse.tile import add_dep_helper
from concourse.bass_utils import run_bass_kernel_spmd

BF16 = ml_dtypes.bfloat16

B = 8
MEM_LEN = 512
D = 32            # decoder steps
K = 512           # input hidden (MEM_H == DEC_H)
H = 1024          # projection hidden
M1 = MEM_LEN + 1  # 513 = terminate slot + mem positions
MPAD = M1 + 1     # 514, even for DVE packing
NCH = H // 128    # 8 h-chunks
NKB = K // 128    # 4 contraction chunks
G = 16            # d's per tanh instruction / per DVE group-add
NEG_BIG = -3.0e38

AF = mybir.ActivationFunctionType
dt = mybir.dt
ALU = mybir.AluOpType

LAST_RESULT = None
_CACHED_NC = None


def _build_bass():
    nc = bacc.Bacc("TRN2", target_bir_lowering=False, debug=False)

    # Per-core inputs (host pre-transposed / pre-cast).
    memT = nc.declare_dram_parameter("memT", [128, NKB * M1], dt.bfloat16, isOutput=False)
    decT = nc.declare_dram_parameter("decT", [128, NKB * D], dt.bfloat16, isOutput=False)
    WmT = nc.declare_dram_parameter("WmT", [NCH, 128, NKB * 128], dt.bfloat16, isOutput=False)
    WdT = nc.declare_dram_parameter("WdT", [NCH, 128, NKB * 128], dt.bfloat16, isOutput=False)
    bsum = nc.declare_dram_parameter("bsum", [128, NCH], dt.float32, isOutput=False)
    ZW = nc.declare_dram_parameter("ZW", [128, NCH * 63], dt.bfloat16, isOutput=False)
    maskB = nc.declare_dram_parameter("maskB", [D, M1], dt.uint8, isOutput=False)
    eye = nc.declare_dram_parameter("eye", [D, D], dt.bfloat16, isOutput=False)
    out = nc.declare_dram_parameter("out", [D, M1], dt.float32, isOutput=True)

    with tile.TileContext(nc) as tc, ExitStack() as ctx:
        consts = ctx.enter_context(tc.tile_pool(name="consts", bufs=1))
        targ_pool = ctx.enter_context(tc.tile_pool(name="targ", bufs=2))
        tout_pool = ctx.enter_context(tc.tile_pool(name="tout", bufs=3))
        ep_pool = ctx.enter_context(tc.tile_pool(name="ep", bufs=1))
        pm_psum = ctx.enter_context(tc.tile_pool(name="pmps", bufs=2, space="PSUM"))
        pd_psum = ctx.enter_context(tc.tile_pool(name="pdps", bufs=2, space="PSUM"))
        sc_psum = ctx.enter_context(tc.tile_pool(name="scps", bufs=1, space="PSUM"))
        s5_psum = ctx.enter_context(tc.tile_pool(name="s5ps", bufs=1, space="PSUM"))

        WmT_sb = [consts.tile([128, NKB * 128], dt.bfloat16, tag=f"WmT{c}", name=f"WmT{c}") for c in range(NCH)]
        WdT_sb = [consts.tile([128, NKB * 128], dt.bfloat16, tag=f"WdT{c}", name=f"WdT{c}") for c in range(NCH)]

        def load_w(c):
            nc.sync.dma_start(WdT_sb[c][:], WdT[c])
            nc.sync.dma_start(WmT_sb[c][:], WmT[c])

        # ---- early loads (needed by chunk 0) ----
        decT_sb = consts.tile([128, NKB * D], dt.bfloat16, tag="decT")
        nc.sync.dma_start(decT_sb[:], decT[:, :])
        bsum_sb = consts.tile([128, NCH], dt.float32, tag="bsum")
        nc.sync.dma_start(bsum_sb[:], bsum[:, :])
        load_w(0)
        memT_sb = consts.tile([128, NKB * M1], dt.bfloat16, tag="memT")
        for kb in range(NKB):
            nc.sync.dma_start(memT_sb[:, bass.ts(kb, M1)], memT[:, bass.ts(kb, M1)])
        load_w(1)
        ZW_sb = consts.tile([128, NCH * 63], dt.bfloat16, tag="ZW")
        nc.gpsimd.dma_start(ZW_sb[:], ZW[:, :])
        maskB_sb = consts.tile([D, M1], dt.uint8, tag="maskB")
        nc.gpsimd.dma_start(maskB_sb[:], maskB[:, :])
        eye_sb = consts.tile([D, D], dt.bfloat16, tag="eye")
        nc.gpsimd.dma_start(eye_sb[:], eye[:, :])

        # pd values stored as duplicated pairs (pd[d] at cols 2d and 2d+1)
        # so the group-add's in1 AP keeps an innermost step-1 pair dim.
        pdb = consts.tile([128, NCH * D * 2], dt.bfloat16, tag="pdb")
        pm_sb = consts.tile([128, NCH * MPAD], dt.bfloat16, tag="pm")

        chunk_psum = {}

        def compute_chunk_pe(c):
            """pd_T and pm_T matmuls for h-chunk c (PE only)."""
            ps = pd_psum.tile([128, D], dt.float32, tag="pdps", name=f"pdps{c}")
            for kb in range(NKB):
                nc.tensor.matmul(
                    ps[:],
                    WdT_sb[c][:, bass.ts(kb, 128)],
                    decT_sb[:, bass.ts(kb, D)],
                    start=(kb == 0),
                    stop=(kb == NKB - 1),
                )
            pp = pm_psum.tile([128, M1], dt.float32, tag="pmps", name=f"pmps{c}")
            for kb in range(NKB):
                lhs = WmT_sb[c][:, bass.ts(kb, 128)]
                nc.tensor.matmul(
                    pp[:, 0:512],
                    lhs,
                    memT_sb[:, kb * M1 : kb * M1 + 512],
                    start=(kb == 0),
                    stop=(kb == NKB - 1),
                    skip_group_check=True,
                )
                nc.tensor.matmul(
                    pp[:, 512:513],
                    lhs,
                    memT_sb[:, kb * M1 + 512 : kb * M1 + 513],
                    start=(kb == 0),
                    stop=(kb == NKB - 1),
                    skip_group_check=True,
                )
            chunk_psum[c] = (ps, pp)

        def compute_chunk_dve(c, order_dep=None):
            """psum -> sbuf drains for h-chunk c, ordered after order_dep."""
            ps, pp = chunk_psum.pop(c)
            dup_out = pdb[:, bass.ts(c, 2 * D)].rearrange("p (d two) -> p d two", two=2)
            dup_in = ps[:].unsqueeze(2).broadcast_to((128, D, 2))
            i1 = nc.vector.tensor_scalar_add(dup_out, dup_in, bsum_sb[:, c : c + 1])
            i2 = nc.vector.tensor_copy(pm_sb[:, c * MPAD : c * MPAD + M1], pp[:])
            nc.gpsimd.memset(pm_sb[:, c * MPAD + M1 : (c + 1) * MPAD], 0.0)
            if order_dep is not None:
                add_dep_helper(i1.ins, order_dep.ins, sync=False, reason="dve order")
                add_dep_helper(i2.ins, order_dep.ins, sync=False, reason="dve order")

        compute_chunk_pe(0)
        compute_chunk_dve(0)

        # ---- main loop: tanh(pm + pd) and weighted reduce over h ----
        # m < 512 goes through sliding one-hot matmuls into score_ps[d, m];
        # the last column (m=512) is gathered per group into s512_ps[0, d].
        score_ps = sc_psum.tile([D, 512], dt.float32, tag="score")
        s512_ps = s5_psum.tile([1, D], dt.float32, tag="s512")
        first = True
        # taper the final chunk so the post-tanh PE tail is short
        groups = [(0, 16), (16, 16)]
        first_groups = [(0, 4), (4, 4), (8, 8), (16, 8), (24, 8)]
        last_groups = [(0, 16), (16, 8), (24, 4), (28, 4)]
        for c in range(NCH):
            pm_c = pm_sb[:, c * MPAD : (c + 1) * MPAD]
            glist = (
                first_groups if c == 0
                else last_groups if c == NCH - 1
                else groups
            )
            for gi, (d0, gsz) in enumerate(glist):
                targ = targ_pool.tile([128, G * MPAD], dt.bfloat16, tag="targ")
                in0 = (
                    pm_c.unsqueeze(1)
                    .broadcast_to((128, gsz, MPAD))
                    .rearrange("p d (a two) -> p d a two", two=2)
                )
                in1 = (
                    pdb[:, c * 2 * D + 2 * d0 : c * 2 * D + 2 * (d0 + gsz)]
                    .rearrange("p (d two) -> p d two", two=2)
                    .unsqueeze(2)
                    .broadcast_to((128, gsz, MPAD // 2, 2))
                )
                tadd = targ[:, 0 : gsz * MPAD].rearrange(
                    "p (d a two) -> p d a two", d=gsz, two=2
                )
                tt_inst = nc.vector.tensor_add(tadd, in0, in1)

                tout = tout_pool.tile([128, G * MPAD], dt.bfloat16, tag="tout")
                nc.scalar.activation(tout[:, 0 : gsz * MPAD], targ[:, 0 : gsz * MPAD], AF.Tanh)
                last = c == NCH - 1 and gi == len(glist) - 1
                if last:
                    g_rhs0 = (
                        tout[:, 0 : gsz * MPAD]
                        .rearrange("p (d m) -> p d m", d=gsz)[:, :, 512:513]
                        .squeeze(2)
                    )
                    nc.tensor.matmul(
                        s512_ps[0:1, d0 : d0 + gsz],
                        ZW_sb[:, c * 63 + 31 : c * 63 + 32],
                        g_rhs0,
                        start=False,
                        stop=True,
                        skip_group_check=True,
                    )
                for j in range(gsz):
                    d = d0 + j
                    lhs = ZW_sb[:, c * 63 + 31 - d : c * 63 + 63 - d]
                    nc.tensor.matmul(
                        score_ps[:, 0:512],
                        lhs,
                        tout[:, j * MPAD : j * MPAD + 512],
                        start=first,
                        stop=(last and j == gsz - 1),
                        skip_group_check=True,
                    )
                    first = False
                # gather the m=512 column of every d in this group at once
                if not last:
                    g_rhs = (
                        tout[:, 0 : gsz * MPAD]
                        .rearrange("p (d m) -> p d m", d=gsz)[:, :, 512:513]
                        .squeeze(2)
                    )
                    nc.tensor.matmul(
                        s512_ps[0:1, d0 : d0 + gsz],
                        ZW_sb[:, c * 63 + 31 : c * 63 + 32],
                        g_rhs,
                        start=(c == 0 and gi == 0),
                        stop=False,
                        skip_group_check=True,
                    )
                if c == 2 and gi == 0:
                    # fold the additive -3e38 mask into the PSUM accumulators
                    maskF = ep_pool.tile([D, M1], dt.bfloat16, tag="maskF")
                    im1 = nc.vector.tensor_scalar_mul(maskF[:], maskB_sb[:], NEG_BIG)
                    mask2 = ep_pool.tile([D, M1], dt.float32, tag="mask2")
                    im2 = nc.vector.tensor_scalar_mul(mask2[:], maskF[:], 2.0)
                    add_dep_helper(im1.ins, tt_inst.ins, sync=False, reason="dve order")
                    add_dep_helper(im2.ins, tt_inst.ins, sync=False, reason="dve order")
                    nc.tensor.matmul(
                        score_ps[:, 0:512], eye_sb[:], maskF[:, 0:512],
                        start=False, stop=False, skip_group_check=True,
                    )
                    nc.tensor.matmul(
                        s512_ps[0:1, :], maskF[:, 512:513], eye_sb[:],
                        start=False, stop=False, skip_group_check=True,
                    )
                # pipeline the next chunk's projections behind this chunk's tanh:
                # PE matmuls early, psum drains late (ordered after this TT)
                if gi == (3 if c == 0 else 0):
                    if c + 2 < NCH:
                        load_w(c + 2)
                    if c + 1 < NCH:
                        compute_chunk_pe(c + 1)
                if gi == (4 if c == 0 else 1):
                    if c + 1 < NCH:
                        compute_chunk_dve(c + 1, order_dep=tt_inst)

        # ---- masked log-softmax along m (mask already folded into PSUM) ----
        # transpose the gathered last column [1,32] -> [32,1]
        tr = ep_pool.tile([32, 32], dt.float32, tag="tr")
        nc.vector.tensor_copy(tr[0:1, :], s512_ps[:])
        tr2 = ep_pool.tile([32, 32], dt.float32, tag="tr2")
        nc.vector.transpose(tr2[:], tr[:])

        nmx = ep_pool.tile([D, 1], dt.float32, tag="nmx")
        nc.vector.tensor_reduce(
            nmx[:], score_ps[:], axis=mybir.AxisListType.X, op=ALU.max, negate=True
        )
        E = ep_pool.tile([D, 512], dt.float32, tag="E")
        Z1 = ep_pool.tile([D, 1], dt.float32, tag="Z1")
        nc.scalar.activation(E[:], score_ps[:], AF.Exp, bias=nmx[:, 0:1], accum_out=Z1[:])
        E2 = ep_pool.tile([D, 1], dt.float32, tag="E2")
        nc.scalar.activation(E2[:], tr2[:, 0:1], AF.Exp, bias=nmx[:, 0:1])
        Z = ep_pool.tile([D, 1], dt.float32, tag="Z")
        nc.vector.tensor_add(Z[:], Z1[:], E2[:])
        # ln(Z) via exponent/mantissa split + degree-5 polynomial on [1,2).
        # Z = sum(exp(S-max)) is always in [1, 514*e^5], so e in [0,17], m in [1,2).
        LN_C = [-1.9367597429421655, 3.514087297000313, -2.440029762614545,
                1.1160900268324192, -0.2838268477821446, 0.030449004538674287]
        zb = Z[:].bitcast(dt.uint32)
        eb = ep_pool.tile([D, 1], dt.uint32, tag="eb")
        nc.vector.tensor_scalar(
            eb[:], zb, 23, None, op0=ALU.logical_shift_right
        )
        ef = ep_pool.tile([D, 1], dt.float32, tag="ef")
        nc.vector.tensor_copy(ef[:], eb[:])  # biased exponent as f32
        mb = ep_pool.tile([D, 1], dt.uint32, tag="mb")
        nc.vector.tensor_scalar(
            mb[:], zb, 0x007FFFFF, 0x3F800000,
            op0=ALU.bitwise_and, op1=ALU.bitwise_or,
        )
        mant = mb[:].bitcast(dt.float32)
        acc = ep_pool.tile([D, 1], dt.float32, tag="acc")
        nc.vector.tensor_scalar(acc[:], mant, LN_C[5], None, op0=ALU.mult)
        for k in (4, 3, 2, 1):
            nc.vector.scalar_tensor_tensor(
                acc[:], acc[:], LN_C[k], mant, ALU.add, ALU.mult
            )
        off = ep_pool.tile([D, 1], dt.float32, tag="off")
        # off = e*ln2 + acc
        nc.vector.scalar_tensor_tensor(
            off[:], ef[:], 0.6931471805599453, acc[:], ALU.mult, ALU.add
        )
        # off2 = (off + c0 - 127*ln2) - nmx = lnZ + mx
        off2 = ep_pool.tile([D, 1], dt.float32, tag="off2")
        nc.vector.scalar_tensor_tensor(
            off2[:], off[:], LN_C[0] - 127 * 0.6931471805599453, nmx[:],
            ALU.add, ALU.subtract
        )
        res2 = ep_pool.tile([D, M1], dt.float32, tag="res2")
        # res2 = (score - off2) + mask2  (exact -inf at masked positions)
        nc.vector.scalar_tensor_tensor(
            res2[:, 0:512], score_ps[:], off2[:, 0:1], mask2[:, 0:512],
            ALU.subtract, ALU.add
        )
        nc.vector.scalar_tensor_tensor(
            res2[:, 512:513], tr2[:, 0:1], off2[:, 0:1], mask2[:, 512:513],
            ALU.subtract, ALU.add
        )
        nc.sync.dma_start(out[:, 0:256], res2[:, 0:256])
        nc.gpsimd.dma_start(out[:, 256:M1], res2[:, 256:M1])

    nc.compile()
    return nc


def _get_nc():
    global _CACHED_NC
    if _CACHED_NC is None:
        _CACHED_NC = _build_bass()
    return _CACHED_NC


def kernel(
    mem,
    dec_hid,
    mem_mask,
    dec_mask,
    dup_mask,
    terminate_state,
    W_mem,
    b_mem,
    W_dec,
    b_dec,
    w_score,
    b_score,
):
    global LAST_RESULT
    mem = np.asarray(mem, np.float32)
    dec_hid = np.asarray(dec_hid, np.float32)
    mem_mask = np.asarray(mem_mask, bool)
    dec_mask = np.asarray(dec_mask, bool)
    dup_mask = np.asarray(dup_mask, bool)
    terminate_state = np.asarray(terminate_state, np.float32)
    W_mem = np.asarray(W_mem, np.float32)
    b_mem = np.asarray(b_mem, np.float32)
    W_dec = np.asarray(W_dec, np.float32)
    b_dec = np.asarray(b_dec, np.float32)
    w_score = np.asarray(w_score, np.float32)
    # b_score shifts every logit equally -> cancels in log_softmax; ignore it.

    b = mem.shape[0]

    # Shared (per-core identical) tensors, in the exact SBUF tile layouts.
    WmT = np.ascontiguousarray(
        W_mem.T.reshape(NKB, 128, NCH, 128).transpose(2, 1, 0, 3).reshape(NCH, 128, NKB * 128)
    ).astype(BF16)
    WdT = np.ascontiguousarray(
        W_dec.T.reshape(NKB, 128, NCH, 128).transpose(2, 1, 0, 3).reshape(NCH, 128, NKB * 128)
    ).astype(BF16)
    bsum = np.ascontiguousarray((b_mem + b_dec).reshape(NCH, 128).T).astype(np.float32)
    EYE = np.eye(D, dtype=np.float32).astype(BF16)
    ZW = np.zeros((128, NCH * 63), BF16)
    for c in range(NCH):
        ZW[:, c * 63 + 31] = w_score[c * 128 : (c + 1) * 128].astype(BF16)

    # Merged boolean mask: True -> -inf.
    mem_mask_full = np.concatenate(
        [np.zeros((b, 1), bool), mem_mask], axis=1
    )  # [B, 513]
    full_mask = (dup_mask & ~dec_mask[:, :, None]) | mem_mask_full[:, None, :]

    in_maps = []
    for i in range(b):
        memT_full = np.empty((K, M1), np.float32)
        memT_full[:, 0] = terminate_state[0]
        memT_full[:, 1:] = mem[i].T
        memTr = memT_full.reshape(NKB, 128, M1).transpose(1, 0, 2).reshape(128, NKB * M1)
        decTr = dec_hid[i].T.reshape(NKB, 128, D).transpose(1, 0, 2).reshape(128, NKB * D)
        in_maps.append(
            {
                "memT": np.ascontiguousarray(memTr).astype(BF16),
                "decT": np.ascontiguousarray(decTr).astype(BF16),
                "WmT": WmT,
                "WdT": WdT,
                "bsum": bsum,
                "ZW": ZW,
                "maskB": full_mask[i].astype(np.uint8),
                "eye": EYE,
            }
        )

    nc = _get_nc()
    trace = bool(os.environ.get("PTR_TRACE"))
    LAST_RESULT = run_bass_kernel_spmd(
        nc, in_maps, list(range(b)), trace=trace
    )
    outs = np.stack([LAST_RESULT.results[i]["out"] for i in range(b)])
    return outs.astype(np.float32)
